# revision 17
# baseline (speedup 1.0000x reference)
"""MoE (dense-act-dense, top-4 of 8 experts) Trainium2 kernel.

Strategy (expert-parallel, host-side dispatch):
  - The forward combine weight is exactly 1.0 (straight-through gate trick in
    the reference), so out[n] = sum_{e in top4(n)} expert_e(x[n]).
  - Host computes the tiny gate matmul + top-4 routing (0.05% of FLOPs) and
    dispatches tokens: core e receives the tokens routed to expert e
    (capacity-padded), plus expert e's weights. This is the sharding step.
  - Each of the 8 cores runs a dense 2-layer MLP (relu between) on its tokens:
      h = relu(w1[e] @ x) ; y = w2[e] @ h
    as two chained bf16 GEMMs (bf16 data, fp32 PSUM accumulate). bf16 halves
    DMA traffic + SBUF vs fp32r at the same 1 cycle/row PE rate, and its
    ~3e-3 rel-err is far inside the 2e-2 gate.
  - Host scatter-adds per-expert outputs back (weight 1.0 per selection).

Per-core device layouts (host pre-arranges everything for contiguous DMA):
  xT  [D, C]  bf16 : routed tokens, transposed (924B runs per partition)
  w1r [H, D]  bf16 : slice-major stationary layout; rows hc*128+k hold
                     w1[e][hc*128+m, dc*128+k] at col dc*128+m, so a 128-col
                     PE slice DMAs as one 4KB-contiguous run per partition.
  w2r [O, H]  bf16 : same trick for layer 2 (2KB runs).
  yT  [O, C]  f32  : expert output, transposed.

Schedule notes:
  - Capacity is exact (max expert load, even-rounded), split into even tile
    widths <= 512 (PSUM bank limit): measured per-mm spacing is ~(NT+10)cyc
    in bf16, so the widest legal tiles minimize the fixed bubble.
  - ~44 dummy matmuls on a memset scratch tile run during the ~12us DMA
    startup window so the PE p-state is fully ramped (2.4GHz) before real
    work; without this the first ~10 real matmuls run at ~half speed.
  - x tiles stream in 4 dc-chunks so the first gemm1 chain starts after
    ~1MB (w1 slice 0 + x0 chunk 0) instead of the full 2.9MB.
  - DMA order on the sync queue: w1s0, x0 (4 chunks), w1s1..7, x1, w2s0..15,
    x2, then one x tile per loop iteration. At ~300GB/s observed, every
    consumer's data lands just ahead of the PE's zero-gap schedule.
  - GEMM2(t) is emitted one tile behind GEMM1(t+1) (depth-1 software
    pipeline) to give the PE GEMM1 work while w2 is still streaming in.
  - y drains: PSUM -> SBUF copy on vector, store DMA issued on scalar, so the
    sync queue (x + weights, latency-critical) is never blocked behind them.
"""

import numpy as np
import ml_dtypes
from contextlib import ExitStack

import concourse.bass as bass
import concourse.tile as tile
from concourse import bacc, mybir
from concourse import bass_utils

F32 = mybir.dt.float32
BF16 = mybir.dt.bfloat16
P = 128

TOP_K = 4
D, H, O, E = 2048, 1024, 2048, 8
_NC_CACHE = {}
NPBF16 = ml_dtypes.bfloat16


def _tile_widths(C, target=512):
    """Split C tokens (padded to even) into even tiles of near-equal width
    <= target (PSUM bank holds 512 fp32)."""
    C = max(C + (C % 2), 256)
    C2 = C // 2
    ntiles = -(-C // target)
    base = C2 // ntiles
    rem = C2 - base * ntiles
    widths = [2 * (base + 1)] * rem + [2 * base] * (ntiles - rem)
    widths.sort(reverse=True)
    assert sum(widths) == C and all(w <= target and w % 2 == 0 for w in widths)
    return widths


def build_expert_kernel(C, target=512):
    """Per-core program: dense [C, D] @ [D, H] -> relu -> @ [H, O] in bf16."""
    DC, HC, OC = D // P, H // P, O // P
    XG = 4  # dc-groups per x tile (chunked DMA)
    widths = _tile_widths(C, target)
    starts = [sum(widths[:i]) for i in range(len(widths))]
    NTILES = len(widths)
    NTMAX = max(widths)
    nc = bacc.Bacc("TRN2", target_bir_lowering=False, debug=False, num_devices=E)
    # x is host-pre-tiled: one dram tensor per tile, laid out [P, DC, w_t] so
    # each partition's slice is one contiguous multi-KB run (the naive [D, C]
    # layout needs 2048 x 924B descriptors per tile and the DMA ring is
    # descriptor-dispatch-bound during startup).
    xR = [
        nc.dram_tensor(f"xR{t}", [P, DC, w], BF16, kind="ExternalInput").ap()
        for t, w in enumerate(widths)
    ]
    w1r = nc.dram_tensor("w1r", [H, D], BF16, kind="ExternalInput").ap()
    w2r = nc.dram_tensor("w2r", [O, H], BF16, kind="ExternalInput").ap()
    yT = nc.dram_tensor("yT", [O, C], F32, kind="ExternalOutput").ap()

    with tile.TileContext(nc) as tc, ExitStack() as ctx:
        dpool = ctx.enter_context(tc.tile_pool(name="d", bufs=1))
        wpool = ctx.enter_context(tc.tile_pool(name="w", bufs=1))
        xpool = ctx.enter_context(tc.tile_pool(name="x", bufs=3 * XG))
        hpool = ctx.enter_context(tc.tile_pool(name="h", bufs=2))
        ypool = ctx.enter_context(tc.tile_pool(name="y", bufs=4))
        psd = ctx.enter_context(tc.tile_pool(name="psd", bufs=1, space="PSUM"))
        ps1 = ctx.enter_context(tc.tile_pool(name="ps1", bufs=2, space="PSUM"))
        ps2 = ctx.enter_context(tc.tile_pool(name="ps2", bufs=4, space="PSUM"))

        # --- PE p-state warmup: dummy matmuls on a zeroed scratch tile fill
        # the DMA startup window so real matmuls start at full clock. Sized
        # to end right at data-ready (~12us): ~0.6us first + ~0.43us each at
        # the 1.2GHz mid p-state. memset on gpsimd (idle, early-booting). ---
        dum = dpool.tile([P, 512], BF16, name="dum")
        nc.gpsimd.memset(dum[:], 0.0)
        pd = psd.tile([P, 512], F32, name="pd")
        NWARM = 7
        for i in range(NWARM):
            nc.tensor.matmul(
                pd[:], dum[:, 0:P], dum[:],
                start=(i == 0), stop=(i == NWARM - 1),
            )

        x_tiles = {}

        def dma_x(t, eng):
            w_t = widths[t]
            G = DC // XG
            chunks = []
            for g in range(XG):
                xc = xpool.tile([P, G, NTMAX], BF16, name="x_t")[:, :, :w_t]
                eng.dma_start(xc[:], xR[t][:, g * G:(g + 1) * G, :])
                chunks.append(xc)
            x_tiles[t] = chunks

        w1s = [None] * HC

        def dma_w1(hc, eng):
            w = wpool.tile([P, DC, P], BF16, name=f"w1s{hc}")
            eng.dma_start(
                w[:],
                w1r[hc * P:(hc + 1) * P, :].rearrange("p (dc j) -> p dc j", dc=DC),
            )
            w1s[hc] = w

        w2s = [None] * OC

        def dma_w2(oc):
            w = wpool.tile([P, HC, P], BF16, name=f"w2s{oc}")
            nc.sync.dma_start(
                w[:],
                w2r[oc * P:(oc + 1) * P, :].rearrange("p (hc j) -> p hc j", hc=HC),
            )
            w2s[oc] = w

        # --- startup DMA stream: ONE fifo (sync queue), strictly ordered by
        # need-time. Splitting across rings splits HBM bandwidth and starves
        # the critical prefix (measured: first chain 9us late on 2 rings). ---
        dma_w1(0, nc.sync)
        dma_x(0, nc.sync)
        for hc in range(1, HC):
            dma_w1(hc, nc.sync)
        if NTILES > 1:
            dma_x(1, nc.sync)
        for oc in range(OC):
            dma_w2(oc)
        if NTILES > 2:
            dma_x(2, nc.sync)

        def gemm1(t):
            w_t = widths[t]
            xc = x_tiles.pop(t)
            h_t = hpool.tile([P, HC, NTMAX], BF16, name="h_t")[:, :, :w_t]
            for hc in range(HC):
                ph = ps1.tile([P, NTMAX], F32, name="ph")[:, :w_t]
                for dc in range(DC):
                    nc.tensor.matmul(
                        ph[:], w1s[hc][:, dc, :], xc[dc // XG][:, dc % XG, :],
                        start=(dc == 0), stop=(dc == DC - 1),
                    )
                nc.scalar.activation(
                    h_t[:, hc, :], ph[:], mybir.ActivationFunctionType.Relu
                )
            return h_t

        def gemm2(t, h_t, last=False):
            w_t = widths[t]
            for oc in range(OC):
                # split the very last chain in half so the post-PE drain
                # (PSUM copy + store) runs on a half-width tile
                splits = [0, w_t // 2 - (w_t // 2) % 2, w_t] \
                    if (last and oc == OC - 1) else [0, w_t]
                for a, b in zip(splits, splits[1:]):
                    po = ps2.tile([P, NTMAX], F32, name="po")[:, :b - a]
                    for hc in range(HC):
                        nc.tensor.matmul(
                            po[:], w2s[oc][:, hc, :], h_t[:, hc, a:b],
                            start=(hc == 0), stop=(hc == HC - 1),
                        )
                    y_t = ypool.tile([P, NTMAX], F32, name="y_t")[:, :b - a]
                    nc.vector.tensor_copy(y_t[:], po[:])
                    nc.scalar.dma_start(
                        yT[oc * P:(oc + 1) * P,
                           starts[t] + a:starts[t] + b], y_t[:]
                    )

        # --- depth-1 software-pipelined main loop: GEMM2 runs one tile
        # behind GEMM1 so the PE has work while w2 streams in at startup ---
        h_tiles = {}
        for t in range(NTILES):
            h_tiles[t] = gemm1(t)
            if t >= 1:
                gemm2(t - 1, h_tiles.pop(t - 1))
            if t + 3 < NTILES:
                dma_x(t + 3, nc.sync)
        gemm2(NTILES - 1, h_tiles.pop(NTILES - 1), last=True)
    nc.compile()
    return nc


def _route(xt, wg):
    """Host-side gate + top-4. Gap between 4th/5th gate values is ~3e-5 for
    this distribution, far above fp32 matmul noise, so fp32 reproduces the
    reference top-k set exactly."""
    gate = xt @ wg  # [N, E] fp32
    top4 = np.argpartition(-gate, TOP_K - 1, axis=1)[:, :TOP_K]  # set, unordered
    return top4


def _w1_slice_major(w1e):
    """[H, D] -> rows hc*128+k, cols dc*128+m = w1e[hc*128+m, dc*128+k]."""
    HC, DC = H // P, D // P
    return np.ascontiguousarray(
        w1e.reshape(HC, P, DC, P).transpose(0, 3, 2, 1).reshape(H, D)
    )


def _w2_slice_major(w2e):
    """[O, H] -> rows oc*128+k, cols hc*128+m = w2e[oc*128+m, hc*128+k]."""
    OC, HC = O // P, H // P
    return np.ascontiguousarray(
        w2e.reshape(OC, P, HC, P).transpose(0, 3, 2, 1).reshape(O, H)
    )


def kernel(x, wg, w1, w2, _want_results=False, _run_kwargs=None):
    x = np.asarray(x, dtype=np.float32)
    wg = np.asarray(wg, dtype=np.float32)
    w1 = np.asarray(w1, dtype=np.float32)
    w2 = np.asarray(w2, dtype=np.float32)
    B, S, Dx = x.shape
    N = B * S
    xt = np.ascontiguousarray(x.reshape(N, Dx))
    top4 = _route(xt, wg)

    # token lists per expert
    sel = np.zeros((N, E), dtype=bool)
    np.put_along_axis(sel, top4, True, axis=1)
    tokens = [np.nonzero(sel[:, e])[0] for e in range(E)]
    counts = np.array([len(t) for t in tokens])
    CAP = max(int(counts.max()), 256)
    CAP += CAP % 2

    if CAP not in _NC_CACHE:
        _NC_CACHE[CAP] = build_expert_kernel(CAP)
    nc = _NC_CACHE[CAP]

    xbf = xt.astype(NPBF16)
    widths = _tile_widths(CAP)
    starts = [sum(widths[:i]) for i in range(len(widths))]
    in_maps = []
    for e in range(E):
        xe = np.zeros((CAP, Dx), dtype=NPBF16)
        xe[:counts[e]] = xbf[tokens[e]]
        im = {
            "w1r": _w1_slice_major(w1[e].astype(NPBF16)),
            "w2r": _w2_slice_major(w2[e].astype(NPBF16)),
        }
        for t, (s0, w) in enumerate(zip(starts, widths)):
            # [P, DC, w]: per-partition contiguous multi-KB DMA runs
            im[f"xR{t}"] = np.ascontiguousarray(
                xe[s0:s0 + w].T.reshape(Dx // 128, 128, w).transpose(1, 0, 2)
            )
        in_maps.append(im)

    res = bass_utils.run_bass_kernel_spmd(
        nc, in_maps, core_ids=list(range(E)), **(_run_kwargs or {})
    )

    out = np.zeros((N, O), dtype=np.float32)
    for e in range(E):
        out[tokens[e]] += res.results[e]["yT"].T[:counts[e]]
    out = out.reshape(B, S, O)
    if _want_results:
        return out, res
    return out
